# revision 25
# baseline (speedup 1.0000x reference)
"""DiagBlockAttention Trainium2 kernel v2 (Bass/Tile, 8 NeuronCores).

Problem (hardcoded from spec nn_DiagBlockAttention):
  x[16, 3136, 768] -> qkv = x @ w_qkv -> 12 heads x 64
  block-local attention over 4x4 spatial blocks (16 tokens each),
  softmax over the 16 tokens of each block per head
  out = attn_out @ w_out + b_out

Sharding: data-parallel over batch, 2 batches per core.

v2 design (vs v1 at 990us):
- ALL matmuls bf16 (rel err ~4e-3 vs 2e-2 gate): FWL weight loads, no
  fused-f32r serial weight load, 2x DVE rates.
- x is block-permuted AND transposed to d-major ON THE HOST, so the
  stage-A PE transposes (24/chunk) vanish; x^T DMAs straight into SBUF.
- Token stream regrouped: per core 392 blocks -> 7 superchunks x 896
  tokens; each superchunk = 7 groups x 128 tokens (8 blocks). All
  attention matmuls use full 128 partitions and 128-col stationaries.
- PV matmul is swapped (stationary = v, moving = P^T) so attention
  output lands d-major; odd heads go to PSUM partitions 64:128 via the
  tile_position col-group (out.base_partition()=64). This kills the
  stage-E PE transposes too.
- Softmax sums via 1-col ones-stationary matmuls into PSUM rows 0/64;
  1/sums is partition-broadcast with a 0-stride-AP DMA, reciprocal'd
  on DVE, and multiplied into o^T d-major (normalization commutes with
  nothing else: it must happen per head before the out projection).
- Out projection consumes o^T directly; bias add doubles as the
  psum->SBUF copy.
"""
import numpy as np
import ml_dtypes
from contextlib import ExitStack

import concourse.bass as bass
import concourse.mybir as mybir
import concourse.tile as tile
from concourse import bacc
from concourse.bass_utils import run_bass_kernel_spmd

# ---- problem constants ----
B, N, DIM = 16, 3136, 768
H, DH = 12, 64
J3 = 3 * H * DH              # 2304
SCALE = DH ** -0.5           # 0.125
NCORES = 8
B_LOC = B // NCORES          # 2
NTOK = B_LOC * N             # 6272 tokens per core
NSC = 7                      # superchunks per core
SC = NTOK // NSC             # 896 tokens per superchunk
NG = SC // 128               # 7 groups of 128 tokens (8 blocks)
KT = DIM // 128              # 6 k-tiles
NHP = H // 2                 # 6 head pairs
# attention spans: groups 0..3 (512 cols) and 4..6 (384 cols)
SPANS = [(0, 4), (4, 3)]     # (first group, ngroups)
F32 = mybir.dt.float32
BF16 = mybir.dt.bfloat16
BFNP = ml_dtypes.bfloat16

_CACHE = {}


def _build():
    nc = bacc.Bacc("TRN2", target_bir_lowering=False, debug=False)

    # host-prepped inputs: x d-major bf16 per superchunk, weights bf16
    x_d = nc.dram_tensor("x", [NSC, DIM, SC], BF16, kind="ExternalInput")
    wqkv_d = nc.dram_tensor("w_qkv", [DIM, J3], BF16, kind="ExternalInput")
    wout_d = nc.dram_tensor("w_out", [DIM, DIM], BF16, kind="ExternalInput")
    bout_d = nc.dram_tensor("b_out", [DIM], F32, kind="ExternalInput")
    # output token-major (block order); host un-permutes
    o_d = nc.dram_tensor("o", [NSC, SC, DIM], F32, kind="ExternalOutput")

    with tile.TileContext(nc) as tc, ExitStack() as ctx:
        const = ctx.enter_context(tc.tile_pool(name="const", bufs=1))
        wpool = ctx.enter_context(tc.tile_pool(name="w", bufs=1))
        xin = ctx.enter_context(tc.tile_pool(name="xin", bufs=2))
        qkp_ = ctx.enter_context(tc.tile_pool(name="qkp", bufs=2))
        vap = ctx.enter_context(tc.tile_pool(name="vap", bufs=2))
        otp = ctx.enter_context(tc.tile_pool(name="otp", bufs=2))
        mid = ctx.enter_context(tc.tile_pool(name="mid", bufs=4))
        outp = ctx.enter_context(tc.tile_pool(name="outp", bufs=3))

        ps_proj = ctx.enter_context(tc.tile_pool(name="ps_proj", bufs=2, space="PSUM"))
        ps_s = ctx.enter_context(tc.tile_pool(name="ps_s", bufs=4, space="PSUM"))
        ps_pv = ctx.enter_context(tc.tile_pool(name="ps_pv", bufs=2, space="PSUM"))

        # ---- input DMAs first (before any preamble compute), so the
        # HBM-bandwidth-bound 7.2MB weight+x load starts immediately.
        # Weights stream in j-chunks ordered by first use: qk-proj only
        # needs w[:, :, 0:128] to start.
        w_sb = wpool.tile([128, KT, J3], BF16)
        wo_sb = wpool.tile([128, KT, DIM], BF16)
        wq_src = wqkv_d.ap().rearrange("(kt p) j -> p kt j", p=128)
        wo_src = wout_d.ap().rearrange("(kt p) j -> p kt j", p=128)
        dma_engs = [nc.sync, nc.scalar, nc.gpsimd]

        def load_xT(sc):
            t = xin.tile([128, KT, SC], BF16, tag="xT")
            src = x_d.ap()[sc].rearrange("(kt p) t -> p kt t", p=128)
            nc.sync.dma_start(t[:, 0:3, :], src[:, 0:3, :])
            nc.gpsimd.dma_start(t[:, 3:6, :], src[:, 3:6, :])
            return t

        xT_next = load_xT(0)

        for i, j0 in enumerate(range(0, J3, 256)):
            dma_engs[i % 3].dma_start(
                w_sb[:, :, j0:j0 + 256], wq_src[:, :, j0:j0 + 256])
        bias1 = const.tile([1, DIM], F32)
        nc.sync.dma_start(bias1[:], bout_d.ap().unsqueeze(0))
        for i, j0 in enumerate(range(0, DIM, 384)):
            dma_engs[i % 3].dma_start(
                wo_sb[:, :, j0:j0 + 384], wo_src[:, :, j0:j0 + 384])

        # ---- constants ----
        # 0/1 block-diag-16 mask, one [128,128] pattern repeated 4x in free
        mask = const.tile([128, 512], BF16)
        nc.gpsimd.memset(mask[:], 1.0)
        mask_v = mask[:].rearrange("p (g b i) -> p g b i", g=4, b=8)
        nc.gpsimd.affine_select(
            out=mask_v, in_=mask_v, compare_op=mybir.AluOpType.is_ge,
            fill=0.0, base=0, pattern=[[0, 4], [-16, 8], [0, 16]],
            channel_multiplier=1)
        nc.gpsimd.affine_select(
            out=mask_v, in_=mask_v, compare_op=mybir.AluOpType.is_ge,
            fill=0.0, base=15, pattern=[[0, 4], [16, 8], [0, 16]],
            channel_multiplier=-1)

        # 64 columns of ones: the sums matmul replicates the softmax
        # denominators across 64 PSUM partitions (same PE cost — the moving
        # stream is what's paid for), making the downstream reciprocal a
        # full-width DVE op with no partition broadcast needed.
        ones64 = const.tile([128, 64], BF16)
        nc.vector.memset(ones64[:], 1.0)

        # bias replicated to 128 partitions via K=1 outer-product matmul
        ones1 = const.tile([1, 128], F32)
        nc.vector.memset(ones1[:], 1.0)
        bias_rep = const.tile([128, DIM], F32)
        for half in range(2):
            bps = ps_proj.tile([128, 384], F32, tag="ps_proj")
            nc.tensor.matmul(bps[:], ones1[:], bias1[:, half * 384:(half + 1) * 384],
                             start=True, stop=True)
            nc.vector.tensor_copy(bias_rep[:, half * 384:(half + 1) * 384], bps[:])

        pending_out = []
        for sc in range(NSC):
            # ---- A: x^T (prefetched one superchunk ahead) ----
            xT = xT_next
            if sc + 1 < NSC:
                xT_next = load_xT(sc + 1)

            # ---- B: q/k projection, d-major [j, t], with the previous
            # superchunk's remaining out-projection groups interleaved ----
            qk = qkp_.tile([128, H, SC], BF16, tag="qk")
            for jt in range(H):
                if pending_out and jt in (4, 7, 10):
                    pending_out.pop(0)()
                for half in range(2):
                    ts = slice(half * 448, (half + 1) * 448)
                    qp = ps_proj.tile([128, 448], F32, tag="ps_proj")
                    for kt in range(KT):
                        nc.tensor.matmul(
                            qp[:], w_sb[:, kt, jt * 128:(jt + 1) * 128],
                            xT[:, kt, ts],
                            start=(kt == 0), stop=(kt == KT - 1))
                    if (2 * jt + half) % 2 == 0:
                        nc.vector.tensor_copy(qk[:, jt, ts], qp[:])
                    else:
                        nc.scalar.copy(qk[:, jt, ts], qp[:])

            va = [vap.tile([128, NG, NHP, DH], BF16, tag=f"va{i}", name=f"va{i}")
                  for i in range(2)]
            oT = otp.tile([128, KT, SC], BF16, tag="oT")

            def emit_v_proj():
                # v projection, token-major, split by head parity:
                # va0[tk, g, hp, dh] = v of head 2hp; va1 = head 2hp+1
                for g in range(NG):
                    for half in range(2):
                        vp = ps_proj.tile([128, 384], F32, tag="ps_proj")
                        for kt in range(KT):
                            nc.tensor.matmul(
                                vp[:], xT[:, kt, g * 128:(g + 1) * 128],
                                w_sb[:, kt, 1536 + half * 384:1536 + (half + 1) * 384],
                                start=(kt == 0), stop=(kt == KT - 1))
                        vv = vp[:].rearrange("p (hp b d) -> p hp b d", hp=3, b=2)
                        hs = slice(3 * half, 3 * half + 3)
                        nc.vector.tensor_copy(va[0][:, g, hs, :], vv[:, :, 0, :])
                        nc.scalar.copy(va[1][:, g, hs, :], vv[:, :, 1, :])

            def emit_scores(hp, span):
                g0, ng = span
                T = ng * 128
                sp = [ps_s.tile([128, T], F32, tag="ps_s", name=f"sp{par}")
                      for par in range(2)]
                # parity-inner: consecutive matmuls hit disjoint PE row
                # groups (0:64 / 64:128), so pairs run concurrently and each
                # LDWEIGHTS overlaps the other parity's in-flight matmul
                for g in range(g0, g0 + ng):
                    gs = slice(g * 128, (g + 1) * 128)
                    ls = slice((g - g0) * 128, (g - g0 + 1) * 128)
                    for par in range(2):
                        rows = slice(64 * par, 64 * par + 64)
                        nc.tensor.matmul(sp[par][:, ls], qk[rows, 6 + hp, gs],
                                         qk[rows, hp, gs], start=True, stop=True)
                pm = []
                for par in range(2):
                    pe_t = mid.tile([128, T], BF16, tag="pexp", name=f"pe{par}")
                    nc.scalar.activation(pe_t[:], sp[par][:],
                                         mybir.ActivationFunctionType.Exp,
                                         scale=SCALE)
                    pmt = mid.tile([128, T], BF16, tag="pm", name=f"pm{par}")
                    nc.vector.tensor_mul(pmt[:], pe_t[:], mask[:, 0:T])
                    pm.append(pmt)
                return pm

            def emit_pv(hp, span, pm):
                g0, ng = span
                T = ng * 128
                po = ps_pv.tile([128, T], F32, tag="ps_pv", name="po")
                ss = ps_s.tile([128, T], F32, tag="ps_s", name="ss")
                for g in range(g0, g0 + ng):
                    ls = slice((g - g0) * 128, (g - g0 + 1) * 128)
                    nc.tensor.matmul(po[0:64, ls], va[0][:, g, hp, :],
                                     pm[0][:, ls], start=True, stop=True)
                    nc.tensor.matmul(po[64:128, ls], va[1][:, g, hp, :],
                                     pm[1][:, ls], start=True, stop=True)
                # sums replicated to partitions 0:64 / 64:128 by the ones64
                # stationary; reciprocal + multiply normalize o^T in place
                nc.tensor.matmul(ss[0:64, :], ones64[:], pm[0][:],
                                 start=True, stop=True)
                nc.tensor.matmul(ss[64:128, :], ones64[:], pm[1][:],
                                 start=True, stop=True)
                rT = mid.tile([128, T], F32, tag="rT")
                nc.vector.reciprocal_approx_fast(rT[:], ss[:])
                nc.vector.tensor_mul(oT[:, hp, g0 * 128:g0 * 128 + T], po[:], rT[:])

            # ---- E (interleaved): out projection + bias, store ----
            def emit_out(g, oT=oT, sc=sc):
                gs = slice(g * 128, (g + 1) * 128)
                ob = outp.tile([128, DIM], F32, tag="out_sb")
                for half in range(2):
                    js = slice(half * 384, (half + 1) * 384)
                    op = ps_proj.tile([128, 384], F32, tag="ps_proj")
                    for kt in range(KT):
                        nc.tensor.matmul(op[:], oT[:, kt, gs], wo_sb[:, kt, js],
                                         start=(kt == 0), stop=(kt == KT - 1))
                    nc.vector.tensor_add(ob[:, js], op[:], bias_rep[:, js])
                nc.sync.dma_start(o_d.ap()[sc, gs], ob[:])

            # Pipeline: the first two units' scores are emitted BEFORE the
            # v projection so their exp->mask chains hide under v-proj
            # matmuls; span0's groups (0..3) become out-projectable once all
            # 6 span0 pvs have run and interleave into span1's pipeline to
            # keep the PE dense through the attention phase.
            units = [(hp, span) for span in SPANS for hp in range(NHP)]
            scored = [emit_scores(*units[0]), emit_scores(*units[1])]
            emit_v_proj()
            for u in range(2, len(units)):
                emit_pv(units[u - 2][0], units[u - 2][1], scored[u - 2])
                if NHP + 2 <= u <= NHP + 5:
                    emit_out(u - NHP - 2)
                scored.append(emit_scores(*units[u]))
            emit_pv(units[-2][0], units[-2][1], scored[-2])
            emit_pv(units[-1][0], units[-1][1], scored[-1])
            # groups 4..6 migrate into the next superchunk's qk-proj phase:
            # their chains (and the last pvs' normalize) drain while the PE
            # streams dense projection matmuls
            if sc + 1 < NSC:
                pending_out = [lambda g=g, f=emit_out: f(g) for g in range(4, NG)]
            else:
                for g in range(4, NG):
                    emit_out(g)

    nc.compile()
    return nc


def _to_stream(x):
    """[B_LOC, 3136, d] raster -> [NTOK, d] block-major stream."""
    b, n, d = x.shape
    x = x.reshape(b, 14, 4, 14, 4, d)          # b, br, ir, bc, ic, d
    x = x.transpose(0, 1, 3, 2, 4, 5)           # b, br, bc, ir, ic, d
    return x.reshape(b * n, d)


def _from_stream(o):
    """inverse of _to_stream: [NTOK, d] -> [B_LOC, 3136, d]."""
    d = o.shape[-1]
    o = o.reshape(B_LOC, 14, 14, 4, 4, d)       # b, br, bc, ir, ic, d
    o = o.transpose(0, 1, 3, 2, 4, 5)           # b, br, ir, bc, ic, d
    return o.reshape(B_LOC, N, d)


def _make_in_maps(x, w_qkv, w_out, b_out):
    x = np.ascontiguousarray(x, dtype=np.float32)
    wq = np.ascontiguousarray(w_qkv, dtype=np.float32).astype(BFNP)
    wo = np.ascontiguousarray(w_out, dtype=np.float32).astype(BFNP)
    bo = np.ascontiguousarray(b_out, dtype=np.float32)
    in_maps = []
    for c in range(NCORES):
        xs = _to_stream(x[c * B_LOC:(c + 1) * B_LOC])      # [6272, 768]
        xT = xs.reshape(NSC, SC, DIM).transpose(0, 2, 1)   # [7, 768, 896]
        xT = np.ascontiguousarray(xT).astype(BFNP)
        in_maps.append({"x": xT, "w_qkv": wq, "w_out": wo, "b_out": bo})
    return in_maps


def kernel(x, w_qkv, w_out, b_out):
    if "nc" not in _CACHE:
        _CACHE["nc"] = _build()
    nc = _CACHE["nc"]

    in_maps = _make_in_maps(x, w_qkv, w_out, b_out)
    res = run_bass_kernel_spmd(nc, in_maps, core_ids=list(range(NCORES)))
    out = np.concatenate(
        [_from_stream(res.results[c]["o"].reshape(NTOK, DIM))
         for c in range(NCORES)], axis=0)
    return out.astype(np.float32)


# revision 26
# speedup vs baseline: 1.0189x; 1.0189x over previous
"""DiagBlockAttention Trainium2 kernel v2 (Bass/Tile, 8 NeuronCores).

Problem (hardcoded from spec nn_DiagBlockAttention):
  x[16, 3136, 768] -> qkv = x @ w_qkv -> 12 heads x 64
  block-local attention over 4x4 spatial blocks (16 tokens each),
  softmax over the 16 tokens of each block per head
  out = attn_out @ w_out + b_out

Sharding: data-parallel over batch, 2 batches per core.

v2 design (vs v1 at 990us):
- ALL matmuls bf16 (rel err ~4e-3 vs 2e-2 gate): FWL weight loads, no
  fused-f32r serial weight load, 2x DVE rates.
- x is block-permuted AND transposed to d-major ON THE HOST, so the
  stage-A PE transposes (24/chunk) vanish; x^T DMAs straight into SBUF.
- Token stream regrouped: per core 392 blocks -> 7 superchunks x 896
  tokens; each superchunk = 7 groups x 128 tokens (8 blocks). All
  attention matmuls use full 128 partitions and 128-col stationaries.
- PV matmul is swapped (stationary = v, moving = P^T) so attention
  output lands d-major; odd heads go to PSUM partitions 64:128 via the
  tile_position col-group (out.base_partition()=64). This kills the
  stage-E PE transposes too.
- Softmax sums via 1-col ones-stationary matmuls into PSUM rows 0/64;
  1/sums is partition-broadcast with a 0-stride-AP DMA, reciprocal'd
  on DVE, and multiplied into o^T d-major (normalization commutes with
  nothing else: it must happen per head before the out projection).
- Out projection consumes o^T directly; bias add doubles as the
  psum->SBUF copy.
"""
import numpy as np
import ml_dtypes
from contextlib import ExitStack

import concourse.bass as bass
import concourse.mybir as mybir
import concourse.tile as tile
from concourse import bacc
from concourse.bass_utils import run_bass_kernel_spmd

# ---- problem constants ----
B, N, DIM = 16, 3136, 768
H, DH = 12, 64
J3 = 3 * H * DH              # 2304
SCALE = DH ** -0.5           # 0.125
NCORES = 8
B_LOC = B // NCORES          # 2
NTOK = B_LOC * N             # 6272 tokens per core
NSC = 7                      # superchunks per core
SC = NTOK // NSC             # 896 tokens per superchunk
NG = SC // 128               # 7 groups of 128 tokens (8 blocks)
KT = DIM // 128              # 6 k-tiles
NHP = H // 2                 # 6 head pairs
# attention spans: groups 0..3 (512 cols) and 4..6 (384 cols)
SPANS = [(0, 4), (4, 3)]     # (first group, ngroups)
F32 = mybir.dt.float32
BF16 = mybir.dt.bfloat16
BFNP = ml_dtypes.bfloat16

_CACHE = {}


def _build():
    nc = bacc.Bacc("TRN2", target_bir_lowering=False, debug=False)

    # host-prepped inputs: x d-major bf16 per superchunk, weights bf16
    x_d = nc.dram_tensor("x", [NSC, DIM, SC], BF16, kind="ExternalInput")
    wqkv_d = nc.dram_tensor("w_qkv", [DIM, J3], BF16, kind="ExternalInput")
    wout_d = nc.dram_tensor("w_out", [DIM, DIM], BF16, kind="ExternalInput")
    bout_d = nc.dram_tensor("b_out", [DIM], F32, kind="ExternalInput")
    # output token-major (block order); host un-permutes
    o_d = nc.dram_tensor("o", [NSC, SC, DIM], F32, kind="ExternalOutput")

    with tile.TileContext(nc) as tc, ExitStack() as ctx:
        const = ctx.enter_context(tc.tile_pool(name="const", bufs=1))
        wpool = ctx.enter_context(tc.tile_pool(name="w", bufs=1))
        xin = ctx.enter_context(tc.tile_pool(name="xin", bufs=2))
        qkp_ = ctx.enter_context(tc.tile_pool(name="qkp", bufs=2))
        vap = ctx.enter_context(tc.tile_pool(name="vap", bufs=2))
        otp = ctx.enter_context(tc.tile_pool(name="otp", bufs=2))
        mid = ctx.enter_context(tc.tile_pool(name="mid", bufs=4))
        outp = ctx.enter_context(tc.tile_pool(name="outp", bufs=3))

        ps_proj = ctx.enter_context(tc.tile_pool(name="ps_proj", bufs=2, space="PSUM"))
        ps_s = ctx.enter_context(tc.tile_pool(name="ps_s", bufs=4, space="PSUM"))
        ps_pv = ctx.enter_context(tc.tile_pool(name="ps_pv", bufs=2, space="PSUM"))

        # ---- input DMAs first (before any preamble compute), so the
        # HBM-bandwidth-bound 7.2MB weight+x load starts immediately.
        # Weights stream in j-chunks ordered by first use: qk-proj only
        # needs w[:, :, 0:128] to start.
        w_sb = wpool.tile([128, KT, J3], BF16)
        wo_sb = wpool.tile([128, KT, DIM], BF16)
        wq_src = wqkv_d.ap().rearrange("(kt p) j -> p kt j", p=128)
        wo_src = wout_d.ap().rearrange("(kt p) j -> p kt j", p=128)
        dma_engs = [nc.sync, nc.scalar, nc.gpsimd]

        def load_xT(sc):
            t = xin.tile([128, KT, SC], BF16, tag="xT")
            src = x_d.ap()[sc].rearrange("(kt p) t -> p kt t", p=128)
            nc.sync.dma_start(t[:, 0:3, :], src[:, 0:3, :])
            nc.gpsimd.dma_start(t[:, 3:6, :], src[:, 3:6, :])
            return t

        # priority order: bias (tiny) and the first w chunk land on queues
        # not carrying x, so sc0's first matmuls start ~12us in; the rest
        # of the 7.2MB streams behind, always ahead of its consumption
        bias1 = const.tile([1, DIM], F32)
        nc.scalar.dma_start(bias1[:], bout_d.ap().unsqueeze(0))
        nc.scalar.dma_start(w_sb[:, :, 0:256], wq_src[:, :, 0:256])
        xT_next = load_xT(0)
        for i, j0 in enumerate(range(256, J3, 256)):
            dma_engs[i % 3].dma_start(
                w_sb[:, :, j0:j0 + 256], wq_src[:, :, j0:j0 + 256])
        for i, j0 in enumerate(range(0, DIM, 384)):
            dma_engs[(i + 1) % 3].dma_start(
                wo_sb[:, :, j0:j0 + 384], wo_src[:, :, j0:j0 + 384])

        # ---- constants ----
        # 0/1 block-diag-16 mask, one [128,128] pattern repeated 4x in free
        mask = const.tile([128, 512], BF16)
        nc.gpsimd.memset(mask[:], 1.0)
        mask_v = mask[:].rearrange("p (g b i) -> p g b i", g=4, b=8)
        nc.gpsimd.affine_select(
            out=mask_v, in_=mask_v, compare_op=mybir.AluOpType.is_ge,
            fill=0.0, base=0, pattern=[[0, 4], [-16, 8], [0, 16]],
            channel_multiplier=1)
        nc.gpsimd.affine_select(
            out=mask_v, in_=mask_v, compare_op=mybir.AluOpType.is_ge,
            fill=0.0, base=15, pattern=[[0, 4], [16, 8], [0, 16]],
            channel_multiplier=-1)

        # 64 columns of ones: the sums matmul replicates the softmax
        # denominators across 64 PSUM partitions (same PE cost — the moving
        # stream is what's paid for), making the downstream reciprocal a
        # full-width DVE op with no partition broadcast needed.
        ones64 = const.tile([128, 64], BF16)
        nc.vector.memset(ones64[:], 1.0)

        # bias replicated to 128 partitions via K=1 outer-product matmul
        ones1 = const.tile([1, 128], F32)
        nc.vector.memset(ones1[:], 1.0)
        bias_rep = const.tile([128, DIM], F32)
        for half in range(2):
            bps = ps_proj.tile([128, 384], F32, tag="ps_proj")
            nc.tensor.matmul(bps[:], ones1[:], bias1[:, half * 384:(half + 1) * 384],
                             start=True, stop=True)
            nc.vector.tensor_copy(bias_rep[:, half * 384:(half + 1) * 384], bps[:])

        pending_out = []
        for sc in range(NSC):
            # ---- A: x^T (prefetched one superchunk ahead) ----
            xT = xT_next
            if sc + 1 < NSC:
                xT_next = load_xT(sc + 1)

            # ---- B: q/k projection, d-major [j, t], with the previous
            # superchunk's remaining out-projection groups interleaved ----
            qk = qkp_.tile([128, H, SC], BF16, tag="qk")
            for jt in range(H):
                if pending_out and jt in (4, 7, 10):
                    pending_out.pop(0)()
                for half in range(2):
                    ts = slice(half * 448, (half + 1) * 448)
                    qp = ps_proj.tile([128, 448], F32, tag="ps_proj")
                    for kt in range(KT):
                        nc.tensor.matmul(
                            qp[:], w_sb[:, kt, jt * 128:(jt + 1) * 128],
                            xT[:, kt, ts],
                            start=(kt == 0), stop=(kt == KT - 1))
                    if (2 * jt + half) % 2 == 0:
                        nc.vector.tensor_copy(qk[:, jt, ts], qp[:])
                    else:
                        nc.scalar.copy(qk[:, jt, ts], qp[:])

            va = [vap.tile([128, NG, NHP, DH], BF16, tag=f"va{i}", name=f"va{i}")
                  for i in range(2)]
            oT = otp.tile([128, KT, SC], BF16, tag="oT")

            def emit_v_proj():
                # v projection, token-major, split by head parity:
                # va0[tk, g, hp, dh] = v of head 2hp; va1 = head 2hp+1
                for g in range(NG):
                    for half in range(2):
                        vp = ps_proj.tile([128, 384], F32, tag="ps_proj")
                        for kt in range(KT):
                            nc.tensor.matmul(
                                vp[:], xT[:, kt, g * 128:(g + 1) * 128],
                                w_sb[:, kt, 1536 + half * 384:1536 + (half + 1) * 384],
                                start=(kt == 0), stop=(kt == KT - 1))
                        vv = vp[:].rearrange("p (hp b d) -> p hp b d", hp=3, b=2)
                        hs = slice(3 * half, 3 * half + 3)
                        nc.vector.tensor_copy(va[0][:, g, hs, :], vv[:, :, 0, :])
                        nc.scalar.copy(va[1][:, g, hs, :], vv[:, :, 1, :])

            def emit_scores(hp, span):
                g0, ng = span
                T = ng * 128
                sp = [ps_s.tile([128, T], F32, tag="ps_s", name=f"sp{par}")
                      for par in range(2)]
                # parity-inner: consecutive matmuls hit disjoint PE row
                # groups (0:64 / 64:128), so pairs run concurrently and each
                # LDWEIGHTS overlaps the other parity's in-flight matmul
                for g in range(g0, g0 + ng):
                    gs = slice(g * 128, (g + 1) * 128)
                    ls = slice((g - g0) * 128, (g - g0 + 1) * 128)
                    for par in range(2):
                        rows = slice(64 * par, 64 * par + 64)
                        nc.tensor.matmul(sp[par][:, ls], qk[rows, 6 + hp, gs],
                                         qk[rows, hp, gs], start=True, stop=True)
                pm = []
                for par in range(2):
                    pe_t = mid.tile([128, T], BF16, tag="pexp", name=f"pe{par}")
                    nc.scalar.activation(pe_t[:], sp[par][:],
                                         mybir.ActivationFunctionType.Exp,
                                         scale=SCALE)
                    pmt = mid.tile([128, T], BF16, tag="pm", name=f"pm{par}")
                    nc.vector.tensor_mul(pmt[:], pe_t[:], mask[:, 0:T])
                    pm.append(pmt)
                return pm

            def emit_pv(hp, span, pm):
                g0, ng = span
                T = ng * 128
                po = ps_pv.tile([128, T], F32, tag="ps_pv", name="po")
                ss = ps_s.tile([128, T], F32, tag="ps_s", name="ss")
                for g in range(g0, g0 + ng):
                    ls = slice((g - g0) * 128, (g - g0 + 1) * 128)
                    nc.tensor.matmul(po[0:64, ls], va[0][:, g, hp, :],
                                     pm[0][:, ls], start=True, stop=True)
                    nc.tensor.matmul(po[64:128, ls], va[1][:, g, hp, :],
                                     pm[1][:, ls], start=True, stop=True)
                # sums replicated to partitions 0:64 / 64:128 by the ones64
                # stationary; reciprocal + multiply normalize o^T in place
                nc.tensor.matmul(ss[0:64, :], ones64[:], pm[0][:],
                                 start=True, stop=True)
                nc.tensor.matmul(ss[64:128, :], ones64[:], pm[1][:],
                                 start=True, stop=True)
                rT = mid.tile([128, T], F32, tag="rT")
                nc.vector.reciprocal_approx_fast(rT[:], ss[:])
                nc.vector.tensor_mul(oT[:, hp, g0 * 128:g0 * 128 + T], po[:], rT[:])

            # ---- E (interleaved): out projection + bias, store ----
            def emit_out(g, oT=oT, sc=sc):
                gs = slice(g * 128, (g + 1) * 128)
                ob = outp.tile([128, DIM], F32, tag="out_sb")
                for half in range(2):
                    js = slice(half * 384, (half + 1) * 384)
                    op = ps_proj.tile([128, 384], F32, tag="ps_proj")
                    for kt in range(KT):
                        nc.tensor.matmul(op[:], oT[:, kt, gs], wo_sb[:, kt, js],
                                         start=(kt == 0), stop=(kt == KT - 1))
                    nc.vector.tensor_add(ob[:, js], op[:], bias_rep[:, js])
                nc.sync.dma_start(o_d.ap()[sc, gs], ob[:])

            # Pipeline: the first two units' scores are emitted BEFORE the
            # v projection so their exp->mask chains hide under v-proj
            # matmuls; span0's groups (0..3) become out-projectable once all
            # 6 span0 pvs have run and interleave into span1's pipeline to
            # keep the PE dense through the attention phase.
            units = [(hp, span) for span in SPANS for hp in range(NHP)]
            scored = [emit_scores(*units[0]), emit_scores(*units[1])]
            emit_v_proj()
            for u in range(2, len(units)):
                emit_pv(units[u - 2][0], units[u - 2][1], scored[u - 2])
                if NHP + 2 <= u <= NHP + 5:
                    emit_out(u - NHP - 2)
                scored.append(emit_scores(*units[u]))
            emit_pv(units[-2][0], units[-2][1], scored[-2])
            emit_pv(units[-1][0], units[-1][1], scored[-1])
            # groups 4..6 migrate into the next superchunk's qk-proj phase:
            # their chains (and the last pvs' normalize) drain while the PE
            # streams dense projection matmuls
            if sc + 1 < NSC:
                pending_out = [lambda g=g, f=emit_out: f(g) for g in range(4, NG)]
            else:
                for g in range(4, NG):
                    emit_out(g)

    nc.compile()
    return nc


def _to_stream(x):
    """[B_LOC, 3136, d] raster -> [NTOK, d] block-major stream."""
    b, n, d = x.shape
    x = x.reshape(b, 14, 4, 14, 4, d)          # b, br, ir, bc, ic, d
    x = x.transpose(0, 1, 3, 2, 4, 5)           # b, br, bc, ir, ic, d
    return x.reshape(b * n, d)


def _from_stream(o):
    """inverse of _to_stream: [NTOK, d] -> [B_LOC, 3136, d]."""
    d = o.shape[-1]
    o = o.reshape(B_LOC, 14, 14, 4, 4, d)       # b, br, bc, ir, ic, d
    o = o.transpose(0, 1, 3, 2, 4, 5)           # b, br, ir, bc, ic, d
    return o.reshape(B_LOC, N, d)


def _make_in_maps(x, w_qkv, w_out, b_out):
    x = np.ascontiguousarray(x, dtype=np.float32)
    wq = np.ascontiguousarray(w_qkv, dtype=np.float32).astype(BFNP)
    wo = np.ascontiguousarray(w_out, dtype=np.float32).astype(BFNP)
    bo = np.ascontiguousarray(b_out, dtype=np.float32)
    in_maps = []
    for c in range(NCORES):
        xs = _to_stream(x[c * B_LOC:(c + 1) * B_LOC])      # [6272, 768]
        xT = xs.reshape(NSC, SC, DIM).transpose(0, 2, 1)   # [7, 768, 896]
        xT = np.ascontiguousarray(xT).astype(BFNP)
        in_maps.append({"x": xT, "w_qkv": wq, "w_out": wo, "b_out": bo})
    return in_maps


def kernel(x, w_qkv, w_out, b_out):
    if "nc" not in _CACHE:
        _CACHE["nc"] = _build()
    nc = _CACHE["nc"]

    in_maps = _make_in_maps(x, w_qkv, w_out, b_out)
    res = run_bass_kernel_spmd(nc, in_maps, core_ids=list(range(NCORES)))
    out = np.concatenate(
        [_from_stream(res.results[c]["o"].reshape(NTOK, DIM))
         for c in range(NCORES)], axis=0)
    return out.astype(np.float32)
